# revision 124
# baseline (speedup 1.0000x reference)
"""BiMPM MatchingLayer kernel for Trainium2, 8 NeuronCores, batch-data-parallel.

Full inputs: p (32,64,600), q (32,64,600), W (8,20,300).
Output: tuple (mv_p, mv_q), each (32,64,160).

Per core: 4 batches x 2 directions (fw: cols 0:300 of p/q, bw: cols 300:600).
All cosine matchings are computed from transposed (h-on-partitions) layouts so
per-row normalizations are per-partition scalars.

v2 "dual-rail" design: every step-indexed (s-on-partitions) tensor keeps the
p-side on partitions 0:64 and the q-side on partitions 64:128 (PSUM col-group
64 via matmul tile_position auto-derive).  Each p/q pair of tail ops then
fuses into ONE 128-partition instruction: norms, sqrt/recip, the maxpool
scale+max-tree+final, and the fu/am/ax combine tails.  The maxpool q-side is a
second (transposed) matmul dlT instead of a partition reduce + DRAM roundtrip;
both dl orientations share one PSUM tile (rows 0:64 / 64:128).  Plain norms
fold into the weighted-norm matmul via a ones column of W^2T.  fu's dens are
broadcast-column matmuls through the shared tail.  Engine placement (cost-model
driven): attentive-max products + all max-trees + the dl scale on DVE (fp16 2x
packed, Pool cannot execute tensor_max and multiplies at 0.42 eff); the W^2
outer-product tensors (via the pre-replicated vtrep, prefetched) and small
squares/copies on Pool; PSUM->SBUF copies, sqrt, C-scaling on ACT; cosine
replication via per-side DMA broadcast reads of a DRAM roundtrip.
"""

import numpy as np

S, H, L, NB, NCORES = 64, 300, 20, 4, 8
CH = [(0, 128), (128, 256), (256, 300)]
WL = 8 * L
WLE = WL + 1

_CACHE = {}


def _bc_mid(bassmod, ap2, n, pos):
    """Insert a stride-0 broadcast dim of count n into a 2D AP's free dims.
    pos=0: (p, f) -> (p, n, f); pos=1: (p, f) -> (p, f, n)."""
    a = list(ap2.ap)
    assert len(a) == 2, a
    if pos == 0:
        new = [a[0], [0, n], a[1]]
    else:
        new = [a[0], a[1], [0, n]]
    return bassmod.AP(tensor=ap2.tensor, offset=ap2.offset, ap=new)


def _build(nb=NB, en=("fu", "mp", "am", "ax"), OFF=300):
    import concourse.bass as bass
    import concourse.tile as tile
    from concourse import bacc, mybir
    from concourse.masks import make_identity
    from contextlib import ExitStack

    f32 = mybir.dt.float32
    bf16 = mybir.dt.float16
    AX = mybir.AxisListType
    OPT = mybir.AluOpType
    ACTF = mybir.ActivationFunctionType

    nc = bacc.Bacc("TRN2", target_bir_lowering=False, debug=False,
                   enable_asserts=False, num_devices=NCORES)
    p_d = nc.dram_tensor("p", [nb, S, 2 * H], f32, kind="ExternalInput").ap()
    q_d = nc.dram_tensor("q", [nb, S, 2 * H], f32, kind="ExternalInput").ap()
    w_d = nc.dram_tensor("W", [8, L, H], f32, kind="ExternalInput").ap()
    op_d = nc.dram_tensor("op", [nb, S, WL], f32, kind="ExternalOutput").ap()
    oq_d = nc.dram_tensor("oq", [nb, S, WL], f32, kind="ExternalOutput").ap()

    with tile.TileContext(nc) as tc, ExitStack() as ctx:
        const = ctx.enter_context(tc.tile_pool(name="const", bufs=1))
        sb = ctx.enter_context(tc.tile_pool(name="sb", bufs=3))
        sbx = ctx.enter_context(tc.tile_pool(name="sbx", bufs=2))
        sbR = ctx.enter_context(tc.tile_pool(name="sbR", bufs=2))
        sbPQ = ctx.enter_context(tc.tile_pool(name="sbPQ", bufs=3))
        sbX = ctx.enter_context(tc.tile_pool(name="sbX", bufs=2))
        sbX2 = ctx.enter_context(tc.tile_pool(name="sbX2", bufs=2))
        sb3 = ctx.enter_context(tc.tile_pool(name="sb3", bufs=3))
        sbO = ctx.enter_context(tc.tile_pool(name="sbO", bufs=1))
        psT = ctx.enter_context(tc.tile_pool(name="psT", bufs=1, space="PSUM"))
        ps = ctx.enter_context(tc.tile_pool(name="ps", bufs=2, space="PSUM"))
        psN = ctx.enter_context(tc.tile_pool(name="psN", bufs=2, space="PSUM"))
        psdl = ctx.enter_context(tc.tile_pool(name="psdl", bufs=1, space="PSUM"))
        dram = ctx.enter_context(tc.tile_pool(name="dram", bufs=8, space="DRAM"))

        ident = const.tile([128, 128], f32, tag="ident")
        make_identity(nc, ident)
        ones = const.tile([1, 128], f32, tag="ones")
        nc.vector.memset(ones[:], 1.0)
        onescf = const.tile([128, 1], f32, tag="onescf")
        nc.gpsimd.memset(onescf[:], 1.0)

        # ---- W precompute: VTall[hp, ci, w*L + l] = W[w, l, h0+hp]^2 ----
        # Column WL holds 1.0 so plain row-norms fold into the p2v matmul.
        vtall = const.tile([128, 3, WLE], f32, tag="vtall")
        nc.gpsimd.memset(vtall[:, :, WL:WLE], 1.0)
        for w in range(8):
            wt = sb.tile([L, H], f32, tag="wt")
            nc.sync.dma_start(wt[:], w_d[w])
            v2 = sb.tile([L, H], f32, tag="v2")
            nc.vector.tensor_mul(v2[:], wt[:], wt[:])
            for ci, (h0, h1) in enumerate(CH):
                hc = h1 - h0
                pt = ps.tile([128, 192], f32, tag="tC")
                nc.tensor.transpose(pt[:hc, 0:L], v2[:, h0:h1], ident[0:L, 0:L])
                nc.scalar.copy(vtall[:hc, ci, w * L:(w + 1) * L], pt[:hc, 0:L])

        vtall16 = const.tile([128, 3, WLE], bf16, tag="vtall16")
        nc.gpsimd.tensor_copy(vtall16[:], vtall[:])

        def vts16(ci, w):
            return vtall16[:CH[ci][1] - CH[ci][0], ci, w * L:(w + 1) * L]

        def vts(ci, w):
            return vtall[:CH[ci][1] - CH[ci][0], ci, w * L:(w + 1) * L]

        # vtrep[h, ci, l, t] = W^2[l,h] replicated across t: keeps the last AP
        # dim packed so the maxpool product tensors run 2x on DVE.
        vtrep = {}
        for w in (2, 3):
            vr = const.tile([128, 3, L, S], bf16, tag=f"vtrep{w}")
            for ci in range(3):
                hc = CH[ci][1] - CH[ci][0]
                base = vtall16[:hc, ci, w * L:(w + 1) * L]
                src = bass.AP(tensor=base.tensor, offset=base.offset,
                              ap=[list(base.ap[0]), [1, L], [0, S]])
                nc.scalar.copy(vr[:hc, ci], src)
            vtrep[w] = vr

        tslot = [0]
        ptbig = psT.tile([128, 8, S], f32, tag="tT")

        def tr_slot():
            s = tslot[0] % 8
            tslot[0] += 1
            return s

        def transpose_to(dst3, dst3bf, src2d, rows, idn):
            """src2d (rows, 300) sbuf -> dst3 (128,3,rows) chunked transpose,
            plus optional bf16 copy into dst3bf."""
            for ci, (h0, h1) in enumerate(CH):
                hc = h1 - h0
                sl = tr_slot()
                nc.tensor.transpose(ptbig[:hc, sl, 0:rows], src2d[:, h0:h1],
                                    idn)
                nc.scalar.copy(dst3[:hc, ci, :], ptbig[:hc, sl, 0:rows])
                if dst3bf is not None:
                    nc.gpsimd.tensor_copy(dst3bf[:hc, ci, :], dst3[:hc, ci, :])

        def flat3(t3, hc, ci):
            """(128,3,A,B) tile -> (hc, A*B) 2D AP for chunk ci."""
            ap = t3[:hc, ci]
            a = list(ap.ap)
            n = 1
            for st, ct in a[1:]:
                n *= ct
            return bass.AP(tensor=ap.tensor, offset=ap.offset, ap=[a[0], [1, n]])

        # tail2: dual-rail mp_cos tail.  numsB (128, L) psum: p rows 0:64,
        # q rows 64:128.  y2 operands are per-ci lhsT APs of the squared
        # attended vectors (128h); fu passes stride-0 broadcast columns.
        def tail2(O, numsB, y2p_ci, y2q_ci, w, invnBB, sg):
            denps = psN.tile([128, 192], f32, tag="tN")
            for ci, (h0, h1) in enumerate(CH):
                hc = h1 - h0
                nc.tensor.matmul(denps[0:S, 0:L], y2p_ci(ci, hc), vts16(ci, w),
                                 start=(ci == 0), stop=(ci == 2))
            for ci, (h0, h1) in enumerate(CH):
                hc = h1 - h0
                nc.tensor.matmul(denps[64:128, 0:L], y2q_ci(ci, hc), vts16(ci, w),
                                 start=(ci == 0), stop=(ci == 2))
            ny = sb.tile([128, L], f32, tag="ny2")
            nc.scalar.sqrt(ny[:], denps[:, 0:L])
            invy = sb.tile([128, L], f32, tag="invy2")
            nc.vector.reciprocal_approx_fast(invy[:], ny[:])
            c1 = sb.tile([128, L], f32, tag="c12")
            nc.vector.tensor_mul(c1[:], invnBB[:, w * L:(w + 1) * L], invy[:])
            c2 = sb.tile([128, L], f32, tag="c22")
            nc.vector.tensor_scalar_min(c2[:], c1[:], 1e8)
            if sg is not None:
                nc.vector.scalar_tensor_tensor(
                    out=O[:, w * L:(w + 1) * L], in0=numsB[:, 0:L],
                    scalar=sg[:], in1=c2[:], op0=OPT.mult, op1=OPT.mult)
            else:
                nc.vector.tensor_mul(O[:, w * L:(w + 1) * L],
                                     numsB[:, 0:L], c2[:])

        def sq_ci(tile3):
            """Square a (128,3,S) transposed tile -> per-ci lhsT slicer."""
            y2 = sb.tile([128, 3, S], bf16, tag="y2t")
            nc.gpsimd.tensor_mul(y2[:], tile3[:], tile3[:])
            return lambda ci, hc: y2[:hc, ci, :]

        def colbc_ci(tile3, tidx):
            """Per-ci lhsT slicer broadcasting column tidx across S rows."""
            def f(ci, hc):
                base = tile3[:hc, ci, tidx:tidx + 1]
                return bass.AP(tensor=base.tensor, offset=base.offset,
                               ap=[list(base.ap[0]), [0, S]])
            return f

        tree_rr = [0]  # round-robin DVE/Pool for ax max-trees

        # O tiles: rows 0:64 = mv_p, rows 64:128 = mv_q, one per batch.
        O_by_b = {}

        for d in range(2):
            for b in range(nb):
                if d == 0:
                    O = sbO.tile([128, WL], f32, tag=f"O{b}")
                    O_by_b[b] = O
                    if len(en) < 4:
                        nc.gpsimd.memset(O[:], 0.0)
                else:
                    O = O_by_b[b]
                c0 = d * H
                fe = tc.high_priority(offset=OFF)
                fe.__enter__()
                P = sbPQ.tile([S, H], f32, tag="P")
                nc.sync.dma_start(P[:], p_d[b, :, c0:c0 + H])
                Q = sbPQ.tile([S, H], f32, tag="Q")
                nc.sync.dma_start(Q[:], q_d[b, :, c0:c0 + H])

                PT = sb3.tile([128, 3, S], f32, tag="PT")
                PTb = sb3.tile([128, 3, S], bf16, tag="PTb")
                transpose_to(PT, PTb, P, S, ident[0:S, 0:S])
                QT = sb3.tile([128, 3, S], f32, tag="QT")
                QTb = sb3.tile([128, 3, S], bf16, tag="QTb")
                transpose_to(QT, QTb, Q, S, ident[0:S, 0:S])
                PT2 = sb3.tile([128, 3, S], bf16, tag="PT2")
                nc.gpsimd.tensor_mul(PT2[:], PTb[:], PTb[:])
                QT2 = sb3.tile([128, 3, S], bf16, tag="QT2")
                nc.gpsimd.tensor_mul(QT2[:], QTb[:], QTb[:])

                # -- weighted norms (all 8 w) + plain norms (col WL), dual-rail
                pqv = ps.tile([128, 192], f32, tag="tC")
                for ci, (h0, h1) in enumerate(CH):
                    hc = h1 - h0
                    nc.tensor.matmul(pqv[0:S, 0:WLE], PT2[:hc, ci, :],
                                     vtall16[:hc, ci, :],
                                     start=(ci == 0), stop=(ci == 2))
                for ci, (h0, h1) in enumerate(CH):
                    hc = h1 - h0
                    nc.tensor.matmul(pqv[64:128, 0:WLE], QT2[:hc, ci, :],
                                     vtall16[:hc, ci, :],
                                     start=(ci == 0), stop=(ci == 2))
                npw = sb.tile([128, WLE], f32, tag="npw")
                nc.scalar.sqrt(npw[:], pqv[:, 0:WLE])
                invnBB = sb.tile([128, WL], f32, tag="invnBB")
                nc.vector.reciprocal_approx_fast(invnBB[:], npw[:, 0:WL])
                invnPQ = sb.tile([128, 1], f32, tag="invnPQ")
                nc.vector.reciprocal(invnPQ[:], npw[:, WL:WLE])

                # -- cosine matrix: raw dots -> scale by 1/|q| (rows 64:128),
                # transpose, scale by 1/|p|, transpose again.
                psC = ps.tile([128, 192], f32, tag="tC")
                for ci, (h0, h1) in enumerate(CH):
                    hc = h1 - h0
                    nc.tensor.matmul(psC[64:128, 0:S], QTb[:hc, ci, :],
                                     PTb[:hc, ci, :],
                                     start=(ci == 0), stop=(ci == 2))
                ctu = sb.tile([128, S], f32, tag="ctu")
                nc.scalar.activation(ctu[64:128, :], psC[64:128, 0:S],
                                     ACTF.Copy, scale=invnPQ[64:128, :])
                cus = tr_slot()
                nc.tensor.transpose(ptbig[0:S, cus, :], ctu[64:128, :],
                                    ident[64:128, 64:128])
                Cs = sb3.tile([S, S], f32, tag="Cs")
                nc.vector.tensor_scalar_mul(Cs[:], ptbig[0:S, cus, :],
                                            invnPQ[0:S, :])
                cts = tr_slot()
                nc.tensor.transpose(ptbig[0:S, cts, :], Cs[:], ident[0:S, 0:S])
                Ct = sb3.tile([S, S], f32, tag="Ct")
                nc.scalar.copy(Ct[:], ptbig[0:S, cts, :])

                # bf16 cosine copies packed side-by-side for one DMA roundtrip
                CsCtb = sb3.tile([S, 2, S], bf16, tag="CsCtb")
                nc.gpsimd.tensor_copy(CsCtb[:, 0, :], Cs[:])
                nc.gpsimd.tensor_copy(CsCtb[:, 1, :], Ct[:])

                repc = None
                if "ax" in en:
                    cd = dram.tile([2, S, S], bf16, tag="cd")
                    nc.scalar.dma_start(
                        bass.AP(tensor=cd.tensor, offset=cd.offset,
                                ap=[[S, S], [S * S, 2], [1, S]]),
                        CsCtb[:])
                    rep = sbR.tile([128, 2, S, S], bf16, tag="rep")
                    nc.scalar.dma_start(rep[:, 0], bass.AP(
                        tensor=cd.tensor, offset=cd.offset,
                        ap=[[0, 128], [1, S * S]]))
                    nc.scalar.dma_start(rep[:, 1], bass.AP(
                        tensor=cd.tensor, offset=cd.offset + S * S,
                        ap=[[0, 128], [1, S * S]]))
                    repc = rep
                    repmix = sbR.tile([88, S * S], bf16, tag="repmix")
                    nc.scalar.dma_start(repmix[0:44, :], bass.AP(
                        tensor=cd.tensor, offset=cd.offset,
                        ap=[[0, 44], [1, S * S]]))
                    nc.scalar.dma_start(repmix[44:88, :], bass.AP(
                        tensor=cd.tensor, offset=cd.offset + S * S,
                        ap=[[0, 44], [1, S * S]]))
                    stackQP = sb3.tile([88, S], bf16, tag="stackQP")
                    nc.gpsimd.tensor_copy(stackQP[0:44, :], QTb[0:44, 2, :])
                    nc.scalar.dma_start(stackQP[44:88, :], PTb[0:44, 2, :])

                # -- mp prep: inv weighted norms in (l, s)/(l, t) layout,
                # replicated across partitions: rows 0:64 = inv|qw| (for the
                # p-side dl), rows 64:128 = inv|pw| (for the q-side dlT).
                repmp = None
                if "mp" in en:
                    wmp = 2 + d
                    repmp = sbR.tile([128, L * S], bf16, tag="repmp")
                    for half, xT2 in ((0, QT2), (1, PT2)):
                        nvt = ps.tile([128, 192], f32, tag="tC")
                        for ci, (h0, h1) in enumerate(CH):
                            hc = h1 - h0
                            nc.tensor.matmul(nvt[0:L, 0:S], vts16(ci, wmp),
                                             xT2[:hc, ci, :],
                                             start=(ci == 0), stop=(ci == 2))
                        nT = sb.tile([L, S], f32, tag="nT")
                        nc.scalar.sqrt(nT[:], nvt[0:L, 0:S])
                        invT = sb.tile([L, S], f32, tag="invT")
                        nc.vector.reciprocal_approx_fast(invT[:], nT[:])
                        invT16 = sb.tile([L, S], bf16, tag="invT16")
                        nc.gpsimd.tensor_copy(invT16[:], invT[:])
                        scrd = dram.tile([L, S], bf16, tag="nTd")
                        nc.scalar.dma_start(scrd[:], invT16[:])
                        nc.scalar.dma_start(
                            repmp[64 * half:64 * (half + 1), :],
                            bass.AP(tensor=scrd.tensor, offset=scrd.offset,
                                    ap=[[0, S], [1, L * S]]))
                    # product tensors: rhsall (W^2 x Q, for dl) and lhsall
                    # (W^2 x P, for the transposed dlT) -- prefetched.
                    rhsall = sbx.tile([128, 3, L, S], bf16, tag="rhsall")
                    lhsall = sbx.tile([128, 3, L, S], bf16, tag="lhsall")
                    for ci in range(3):
                        hc = CH[ci][1] - CH[ci][0]
                        nc.gpsimd.tensor_mul(rhsall[:hc, ci],
                                             vtrep[wmp][:hc, ci],
                                             _bc_mid(bass, QTb[:hc, ci, :], L, 0))
                        nc.gpsimd.tensor_mul(lhsall[:hc, ci],
                                             vtrep[wmp][:hc, ci],
                                             _bc_mid(bass, PTb[:hc, ci, :], L, 0))
                fe.__exit__(None, None, None)

                # ============ FULL matching (w = d) ============
                if "fu" in en:
                    w = d
                    tidx = S - 1 if d == 0 else 0
                    numsF = psN.tile([128, 192], f32, tag="tN")
                    for half, (xT, yT) in enumerate(((PTb, QTb), (QTb, PTb))):
                        g = sb.tile([128, 3, S], bf16, tag="gf")
                        ylast = bass.AP(tensor=yT.tensor,
                                        offset=yT[:, 0, tidx:tidx + 1].offset,
                                        ap=[list(yT.ap[0]), [S, 3], [0, S]])
                        nc.vector.tensor_mul(g[:], xT[:], ylast)
                        r0 = 64 * half
                        for ci, (h0, h1) in enumerate(CH):
                            hc = h1 - h0
                            nc.tensor.matmul(numsF[r0:r0 + S, 0:L], g[:hc, ci, :],
                                             vts16(ci, w),
                                             start=(ci == 0), stop=(ci == 2))
                    # den: |W o last-step| via broadcast-column den matmuls in
                    # the shared tail (p rail sees q_last, q rail p_last).
                    tail2(O, numsF, colbc_ci(QT2, tidx), colbc_ci(PT2, tidx),
                          w, invnBB, None)

                # ============ MAXPOOL matching (w = 2 + d) ============
                if "mp" in en:
                    w = 2 + d
                    # dl rows 0:64: dl[s,(l,t)]; rows 64:128: dlT[t,(l,s)]
                    dlb = psdl.tile([128, L * S], f32, tag="dl")
                    for ci, (h0, h1) in enumerate(CH):
                        hc = h1 - h0
                        rh_q = flat3(rhsall, hc, ci)
                        rh_p = flat3(lhsall, hc, ci)
                        for (n0, n1) in ((0, 512), (512, 1024), (1024, 1280)):
                            nc.tensor.matmul(dlb[0:S, n0:n1], PTb[:hc, ci, :],
                                             rh_q[:, n0:n1],
                                             start=(ci == 0), stop=(ci == 2))
                            nc.tensor.matmul(dlb[64:128, n0:n1], QTb[:hc, ci, :],
                                             rh_p[:, n0:n1],
                                             start=(ci == 0), stop=(ci == 2))
                    dsc0 = sb.tile([128, L * S], bf16, tag="dsc0")
                    nc.scalar.copy(dsc0[:], dlb[:])
                    # scale both rails by the replicated inverse norms (in
                    # place), then one fused max-tree over the innermost axis.
                    dsc = bass.AP(tensor=dsc0.tensor, offset=dsc0.offset,
                                  ap=[list(dsc0.ap[0]), [S, L], [1, S]])
                    rep3 = bass.AP(tensor=repmp.tensor, offset=repmp.offset,
                                   ap=[list(repmp.ap[0]), [S, L], [1, S]])
                    nc.vector.tensor_mul(dsc, dsc, rep3)

                    def dscv(t0, tn):
                        return bass.AP(tensor=dsc0.tensor,
                                       offset=dsc0.offset + t0,
                                       ap=[list(dsc0.ap[0]), [S, L], [1, tn]])

                    hw_ = 32
                    while hw_ >= 1:
                        nc.vector.tensor_max(dscv(0, hw_), dscv(0, hw_),
                                             dscv(hw_, hw_))
                        hw_ //= 2
                    d0 = bass.AP(tensor=dsc0.tensor, offset=dsc0.offset,
                                 ap=[list(dsc0.ap[0]), [S, L]])
                    nc.vector.tensor_mul(O[:, w * L:(w + 1) * L], d0,
                                         invnBB[:, w * L:(w + 1) * L])

                # ============ ATTENTIVE-MEAN matching (w = 4 + d) ============
                if "am" in en:
                    w = 4 + d
                    # row sums of Cs -> rows 0:64; row sums of Ct -> 64:128
                    rcs = psN.tile([128, 192], f32, tag="tN")
                    nc.tensor.matmul(rcs[0:S, 0:1], Ct[:], onescf[0:S, :],
                                     start=True, stop=True)
                    nc.tensor.matmul(rcs[64:128, 0:1], Cs[:], onescf[0:S, :],
                                     start=True, stop=True)
                    sg = sb.tile([128, 1], f32, tag="sg")
                    nc.scalar.sign(sg[:], rcs[:, 0:1])

                    yv = {}
                    for (nm, nat, cmat) in (("p", Q, Ct), ("q", P, Cs)):
                        yvu = psN.tile([128, 192], f32, tag="tN")
                        for ci, (h0, h1) in enumerate(CH):
                            hc = h1 - h0
                            nc.tensor.matmul(yvu[:hc, ci * S:(ci + 1) * S],
                                             nat[:, h0:h1], cmat[:],
                                             start=True, stop=True)
                        yvs = sb.tile([128, 3, S], bf16, tag="yvs" + nm)
                        nc.scalar.copy(yvs[:], bass.AP(
                            tensor=yvu.tensor, offset=yvu.offset,
                            ap=[list(yvu.ap[0]), [S, 3], [1, S]]))
                        yv[nm] = yvs
                    numsB = psN.tile([128, 192], f32, tag="tN")
                    for half, (nm, statTb) in enumerate((("p", PTb), ("q", QTb))):
                        g = sb.tile([128, 3, S], bf16, tag="gam" + nm)
                        nc.gpsimd.tensor_mul(g[:], statTb[:], yv[nm][:])
                        r0 = 64 * half
                        for ci, (h0, h1) in enumerate(CH):
                            hc = h1 - h0
                            nc.tensor.matmul(numsB[r0:r0 + S, 0:L],
                                             g[:hc, ci, :], vts16(ci, w),
                                             start=(ci == 0), stop=(ci == 2))
                    tail2(O, numsB, sq_ci(yv["p"]), sq_ci(yv["q"]), w,
                          invnBB, sg)

                # ============ ATTENTIVE-MAX matching (w = 6 + d) ============
                if "ax" in en:
                    w = 6 + d
                    ymaxs = {}
                    for (nm, sidx, srcTb) in (("q", 0, QTb), ("p", 1, PTb)):
                        ymaxT = sb3.tile([128, 3, S], bf16, tag="ymaxT" + nm)
                        ymaxs[nm] = ymaxT
                        X = sbX.tile([128, 2, S, S], bf16, tag="X")
                        sb_ = srcTb[:, 0:2, :]
                        in0 = bass.AP(tensor=sb_.tensor, offset=sb_.offset,
                                      ap=[list(sb_.ap[0]), [S, 2], [0, S],
                                          [1, S]])
                        r_ = repc[:, sidx]
                        in1 = bass.AP(tensor=r_.tensor, offset=r_.offset,
                                      ap=[list(r_.ap[0]), [0, 2], [S, S],
                                          [1, S]])
                        nc.vector.tensor_mul(X[:], in0, in1)
                        hw_ = 32
                        while hw_ >= 2:
                            nc.vector.tensor_max(X[:, :, :, 0:hw_],
                                                 X[:, :, :, 0:hw_],
                                                 X[:, :, :, hw_:2 * hw_])
                            hw_ //= 2
                        ym3 = bass.AP(tensor=ymaxT.tensor,
                                      offset=ymaxT[:, 0, :].offset,
                                      ap=[list(ymaxT.ap[0]), [S, 2], [1, S],
                                          [1, 1]])
                        nc.vector.tensor_max(ym3, X[:, :, :, 0:1],
                                             X[:, :, :, 1:2])
                    # fused ci2: both sides' 44-partition chunks at once
                    X2 = sbX2.tile([88, S, S], bf16, tag="X2")
                    in0 = _bc_mid(bass, stackQP[:], S, 0)
                    rm3 = bass.AP(tensor=repmix.tensor, offset=repmix.offset,
                                  ap=[list(repmix.ap[0]), [S, S], [1, S]])
                    nc.vector.tensor_mul(X2[:], in0, rm3)
                    hw_ = 32
                    while hw_ >= 2:
                        nc.vector.tensor_max(X2[:, :, 0:hw_], X2[:, :, 0:hw_],
                                             X2[:, :, hw_:2 * hw_])
                        hw_ //= 2
                    ym2 = sb3.tile([88, S], bf16, tag="ym2")
                    ym23 = bass.AP(tensor=ym2.tensor, offset=ym2.offset,
                                   ap=[list(ym2.ap[0]), [1, S], [1, 1]])
                    nc.vector.tensor_max(ym23, X2[:, :, 0:1], X2[:, :, 1:2])
                    nc.gpsimd.tensor_copy(ymaxs["q"][0:44, 2, :], ym2[0:44, :])
                    nc.scalar.dma_start(ymaxs["p"][0:44, 2, :], ym2[44:88, :])

                    numsB = psN.tile([128, 192], f32, tag="tN")
                    for half, (nm, statTb) in enumerate((("p", PTb), ("q", QTb))):
                        g = sb.tile([128, 3, S], bf16, tag="gax" + nm)
                        src = ymaxs["q"] if nm == "p" else ymaxs["p"]
                        nc.gpsimd.tensor_mul(g[:], statTb[:], src[:])
                        r0 = 64 * half
                        for ci, (h0, h1) in enumerate(CH):
                            hc = h1 - h0
                            nc.tensor.matmul(numsB[r0:r0 + S, 0:L],
                                             g[:hc, ci, :], vts16(ci, w),
                                             start=(ci == 0), stop=(ci == 2))
                    tail2(O, numsB, sq_ci(ymaxs["q"]), sq_ci(ymaxs["p"]), w,
                          invnBB, None)

                if d == 1:
                    nc.sync.dma_start(op_d[b], O[0:S, :])
                    nc.sync.dma_start(oq_d[b], O[64:128, :])



    nc.compile()
    return nc


def _get_nc(nb=NB, en=("fu", "mp", "am", "ax")):
    key = (nb, tuple(en))
    if key not in _CACHE:
        _CACHE[key] = _build(nb, en)
    return _CACHE[key]


def _run(p, q, W, nb=NB, en=("fu", "mp", "am", "ax"), trace=False):
    from concourse.bass_utils import run_bass_kernel_spmd
    nc = _get_nc(nb, en)
    B = p.shape[0]
    ncores = B // nb
    assert ncores == NCORES and B == nb * NCORES
    in_maps = []
    for c in range(NCORES):
        in_maps.append({
            "p": np.ascontiguousarray(p[c * nb:(c + 1) * nb]),
            "q": np.ascontiguousarray(q[c * nb:(c + 1) * nb]),
            "W": np.ascontiguousarray(W),
        })
    r = run_bass_kernel_spmd(nc, in_maps, core_ids=list(range(NCORES)), trace=trace)
    if trace:
        print("HW exec time:", r.exec_time_ns, "ns")
        print("trace:", r.instructions_and_trace[1] if r.instructions_and_trace else None)
    mv_p = np.concatenate([r.results[c]["op"] for c in range(NCORES)], axis=0)
    mv_q = np.concatenate([r.results[c]["oq"] for c in range(NCORES)], axis=0)
    return mv_p, mv_q


def kernel(p, q, W):
    p = np.asarray(p, dtype=np.float32)
    q = np.asarray(q, dtype=np.float32)
    W = np.asarray(W, dtype=np.float32)
    return _run(p, q, W)
